# revision 36
# baseline (speedup 1.0000x reference)
"""Trainium2 Bass kernel for Bahdanau-style attention pooling.

Reference computation (per batch b):
    qp   = q @ Wy + by                         # [B,H]
    h    = tanh(v @ Wx + bx + qp[:,None,:])    # [B,R,H]
    l    = h @ Wa + ba                         # [B,R,1]
    p    = softmax(l, axis=R)                  # [B,R,1]  (ba cancels)
    vw   = sum(p * v, axis=R)                  # [B,Dv]
    returns (vw, p)

Strategy: data-parallel over batch across 8 NeuronCores (8 batches/core).
Single pass over v from HBM per core (64 MiB fp32): v is cast to fp16
during the SWDGE load and xbar-transposed on-chip for the v@Wx contraction
(which needs D on partitions).  Matmuls in fp16 with fp32 PSUM
accumulation; qp/softmax math in fp32.  The probs-weighted sum runs off
the TensorEngine: probs are broadcast across partitions with a K=1 ones
matmul, then the VectorEngine multiplies v^T tiles by them and reduces
along rows.  Softmax skips max-subtraction (logits are O(1) by
construction: tanh in [-1,1], Wa ~ N(0,1/H)).
"""

import numpy as np

import concourse.bacc as bacc
import concourse.tile as tile
import concourse.mybir as mybir
from concourse.bass_utils import run_bass_kernel_spmd

F32 = mybir.dt.float32
F16 = mybir.dt.float16
AFT = mybir.ActivationFunctionType

NCORES = 8
B = 64            # total batch
BPC = B // NCORES  # batches per core
R = 2048          # regions
DV = 1024         # v feature dim
DQ = 1024         # q feature dim
H = 512           # hidden
P = 128           # partitions
DC = DV // P      # 8 d-chunks
HC = H // P       # 4 h-chunks
RT = 4            # row tiles per batch (512 rows each)
ROWS = R // RT    # 512
RCH = ROWS // P   # 4 row chunks per row tile


def build_kernel(tc, ctx, reps=1, loop_n=0, skip=()):
    nc = tc.nc
    v_s = nc.dram_tensor("v_s", [BPC, R, DV], F32, kind="ExternalInput").ap()
    q_s = nc.dram_tensor("q_s", [BPC, DQ], F32, kind="ExternalInput").ap()
    Wx = nc.dram_tensor("Wx", [DV, H], F32, kind="ExternalInput").ap()
    bx = nc.dram_tensor("bx", [H], F32, kind="ExternalInput").ap()
    Wy = nc.dram_tensor("Wy", [DQ, H], F32, kind="ExternalInput").ap()
    by = nc.dram_tensor("by", [H], F32, kind="ExternalInput").ap()
    Wa = nc.dram_tensor("Wa", [H, 1], F32, kind="ExternalInput").ap()
    vw_s = nc.dram_tensor("vw_s", [BPC, DV], F32, kind="ExternalOutput").ap()
    probs_s = nc.dram_tensor("probs_s", [BPC, R], F32, kind="ExternalOutput").ap()

    consts = ctx.enter_context(tc.tile_pool(name="consts", bufs=1))

    # ---- constants into SBUF ----
    wx_bf = consts.tile([P, DC, H], F16, tag="wx")
    nc.gpsimd.dma_start(wx_bf[:], Wx.rearrange("(dc p) h -> p dc h", p=P))
    wa_bf = consts.tile([P, HC], F16, tag="wa")
    nc.gpsimd.dma_start(wa_bf[:], Wa.rearrange("(hc p) one -> p (hc one)", p=P))
    qT = consts.tile([P, DC, BPC], F32, tag="qT")
    for dc in range(DC):
        nc.sync.dma_start(
            qT[:, dc, :], q_s[:, dc * P:(dc + 1) * P].rearrange("b p -> p b"))
    bxT = consts.tile([P, HC], F32, tag="bxT")
    nc.sync.dma_start(bxT[:], bx.rearrange("(hc p) -> p hc", p=P))
    byT = consts.tile([P, HC], F32, tag="byT")
    nc.sync.dma_start(byT[:], by.rearrange("(hc p) -> p hc", p=P))
    bb = consts.tile([P, HC], F32, tag="bb")
    nc.vector.tensor_add(bb[:], bxT[:], byT[:])
    ones16 = consts.tile([1, P], F16, tag="ones16")
    nc.vector.memset(ones16[:], 1.0)

    # biasT[:, hc, b] = (q_b @ Wy)[hc-chunk] + by[hc-chunk] + bx[hc-chunk]
    biasT = consts.tile([P, HC, BPC], F32, tag="biasT")
    with tc.tile_pool(name="qp_psum", bufs=HC, space="PSUM") as qp_pool, \
         tc.tile_pool(name="wy", bufs=1) as wy_pool:
        wy_sb = wy_pool.tile([P, DC, H], F32, tag="wy")
        nc.sync.dma_start(wy_sb[:], Wy.rearrange("(dc p) h -> p dc h", p=P))
        for hc in range(HC):
            qp_ps = qp_pool.tile([P, BPC], F32, tag="qp", name=f"qp{hc}")
            for dc in range(DC):
                nc.tensor.matmul(
                    qp_ps[:],
                    lhsT=wy_sb[:, dc, hc * P:(hc + 1) * P],
                    rhs=qT[:, dc, :],
                    start=(dc == 0),
                    stop=(dc == DC - 1),
                )
            nc.scalar.add(biasT[:, hc, :], qp_ps[:], bb[:, hc:hc + 1])

    # ---- pools for the main loop ----
    vnat_pool = ctx.enter_context(tc.tile_pool(name="vnat", bufs=3))
    vT_pool = ctx.enter_context(tc.tile_pool(name="vT", bufs=RT + 3))
    h_pool = ctx.enter_context(tc.tile_pool(name="h", bufs=2 * HC))
    pre_pool = ctx.enter_context(tc.tile_pool(name="pre", bufs=4, space="PSUM"))
    lpsum_pool = ctx.enter_context(tc.tile_pool(name="lpsum", bufs=2, space="PSUM"))
    bc_pool = ctx.enter_context(tc.tile_pool(name="bc", bufs=2, space="PSUM"))
    rows_pool = ctx.enter_context(tc.tile_pool(name="rows", bufs=2))

    pending_logits = None  # (h_tiles, rt, logits_row)
    pending_wsum = None    # closure

    def flush_logits():
        nonlocal pending_logits
        if pending_logits is None:
            return
        h_tiles, rt, logits_row = pending_logits
        pending_logits = None
        psum_l = lpsum_pool.tile([1, ROWS], F32, tag="lp")
        for hc in range(HC):
            nc.tensor.matmul(
                psum_l[:],
                lhsT=wa_bf[:, hc:hc + 1],
                rhs=h_tiles[hc][:],
                start=(hc == 0),
                stop=(hc == HC - 1),
            )
        nc.scalar.copy(logits_row[:, rt * ROWS:(rt + 1) * ROWS], psum_l[:])

    def make_wsum(b, vTs_b, probsrow16):
        def emit():
            # broadcast probs across partitions via K=1 ones matmul (PE),
            # then weighted sum off-PE: DVE mult + DVE row-reduce.
            probsBC = rows_pool.tile([P, RT, RCH, P], F16, tag="probsBC",
                                     name=f"pbc{b}")
            for rt in range(RT):
                bc_ps = bc_pool.tile([P, ROWS], F32, tag="bc",
                                     name=f"bc{b}_{rt}")
                nc.tensor.matmul(
                    bc_ps[:], lhsT=ones16[:],
                    rhs=probsrow16[:, rt * ROWS:(rt + 1) * ROWS],
                    start=True, stop=True)
                nc.scalar.copy(probsBC[:, rt], bc_ps[:])
            vwT = rows_pool.tile([P, DC], F32, tag="vwT", name=f"vwT{b}")
            scr = rows_pool.tile([P, RT, RCH, P], F16, tag="wscr",
                                 name=f"wscr{b}")
            for dc in range(DC):
                for rt in range(RT):
                    nc.vector.tensor_tensor(
                        out=scr[:, rt],
                        in0=vTs_b[rt][:, :, dc, :],
                        in1=probsBC[:, rt],
                        op=mybir.AluOpType.mult)
                nc.vector.reduce_sum(vwT[:, dc:dc + 1], scr[:],
                                     axis=mybir.AxisListType.XYZ)
            nc.scalar.dma_start(vw_s[b].rearrange("(dc p) -> p dc", p=P),
                                vwT[:])
        return emit

    def batch_body(b):
        nonlocal pending_logits, pending_wsum
        logits_row = rows_pool.tile([1, R], F32, tag="logits_row",
                                    name=f"lr{b}")
        vTs_b = []
        for rt in range(RT):
            # load + cast fp32 -> fp16 (SWDGE), natural layout
            vnat = vnat_pool.tile([P, RCH, DV], F16, tag="vnat")
            nc.gpsimd.dma_start(
                vnat[:],
                v_s[b, rt * ROWS:(rt + 1) * ROWS, :].rearrange(
                    "(rc p) d -> p rc d", p=P),
            )
            # transpose each [128 rows, 1024 D] -> [128 D, (dc, 128 rows)]
            vT = vT_pool.tile([P, RCH, DC, P], F16, tag="vT")
            for rc in range(1 if "xbar" in skip else RCH):
                nc.sync.dma_start(vT[:, rc], vnat[:, rc, :], transpose=True)
            vTs_b.append(vT)
            # pre^T[hc] = sum_dc Wx[dc,hc].T @ vT[dc]  (+bias via ACT)
            h_tiles = []
            for hc in range(HC):
                pre = pre_pool.tile([P, ROWS], F32, tag="pre")
                ndc = 1 if "mm1" in skip else DC
                for dc in range(ndc):
                    nc.tensor.matmul(
                        pre[:],
                        lhsT=wx_bf[:, dc, hc * P:(hc + 1) * P],
                        rhs=vT[:, :, dc, :],
                        start=(dc == 0),
                        stop=(dc == ndc - 1),
                    )
                h_t = h_pool.tile([P, ROWS], F16, tag="h")
                nc.scalar.activation(h_t[:], pre[:], AFT.Tanh,
                                     bias=biasT[:, hc, b:b + 1])
                h_tiles.append(h_t)
            if rt == 1 and pending_wsum is not None:
                pending_wsum()
                pending_wsum = None
            flush_logits()
            pending_logits = (h_tiles, rt, logits_row)
        flush_logits()

        # ---- softmax over the full row (fp32, no max subtraction) ----
        exp_row = rows_pool.tile([1, R], F32, tag="exp_row")
        ssum = rows_pool.tile([1, 1], F32, tag="ssum")
        nc.scalar.activation(exp_row[:], logits_row[:], AFT.Exp,
                             accum_out=ssum[:])
        rinv = rows_pool.tile([1, 1], F32, tag="rinv")
        nc.vector.reciprocal(rinv[:], ssum[:])
        probs_row = rows_pool.tile([1, R], F32, tag="probs_row")
        nc.scalar.mul(probs_row[:], exp_row[:], rinv[:])
        nc.scalar.dma_start(probs_s[b], probs_row[:])
        probsrow16 = rows_pool.tile([1, R], F16, tag="probsrow16")
        nc.scalar.copy(probsrow16[:], probs_row[:])

        pending_wsum = make_wsum(b, vTs_b, probsrow16)

    def all_batches():
        nonlocal pending_wsum
        for b in range(BPC):
            batch_body(b)
        pending_wsum()
        pending_wsum = None

    if loop_n:
        with tc.For_i(0, loop_n, 1):
            all_batches()
    else:
        for _ in range(reps):
            all_batches()


_NC_CACHE = {}


def _get_nc(reps=1, loop_n=0, skip=()):
    key = ("nc", reps, loop_n, tuple(skip))
    if key not in _NC_CACHE:
        from contextlib import ExitStack
        nc = bacc.Bacc("TRN2", target_bir_lowering=False, debug=False,
                       enable_asserts=False, num_devices=NCORES)
        with tile.TileContext(nc) as tc:
            with ExitStack() as ctx:
                build_kernel(tc, ctx, reps=reps, loop_n=loop_n, skip=skip)
        nc.compile()
        _NC_CACHE[key] = nc
    return _NC_CACHE[key]


def kernel(v, q, Wx, bx, Wy, by, Wa, ba):
    nc = _get_nc()
    v = np.ascontiguousarray(np.asarray(v, dtype=np.float32))
    q = np.ascontiguousarray(np.asarray(q, dtype=np.float32))
    Wx = np.ascontiguousarray(np.asarray(Wx, dtype=np.float32))
    bx = np.ascontiguousarray(np.asarray(bx, dtype=np.float32))
    Wy = np.ascontiguousarray(np.asarray(Wy, dtype=np.float32))
    by = np.ascontiguousarray(np.asarray(by, dtype=np.float32))
    Wa = np.ascontiguousarray(np.asarray(Wa, dtype=np.float32))

    in_maps = []
    for c in range(NCORES):
        sl = slice(c * BPC, (c + 1) * BPC)
        in_maps.append({
            "v_s": v[sl], "q_s": q[sl],
            "Wx": Wx, "bx": bx, "Wy": Wy, "by": by, "Wa": Wa,
        })
    res = run_bass_kernel_spmd(nc, in_maps, core_ids=list(range(NCORES)))
    vw = np.concatenate([r["vw_s"] for r in res.results], axis=0)
    probs = np.concatenate([r["probs_s"] for r in res.results], axis=0)
    return vw, probs.reshape(B, R, 1)
